# revision 1
# baseline (speedup 1.0000x reference)
"""Trainium2 Bass kernel for block-local (sliding-window) GQA attention with RoPE.

Module: x:[1,4096,2048] -> Q/K/V proj -> RoPE -> block-local attention
(window W=1024, block 1024, GQA 16 q-heads / 4 kv-heads, D=128) -> out proj.

Sharding: sequence-parallel over 8 cores, 512 queries per core. Each core
recomputes K/V for its 1536-row key span (queries + 1024 history, zero-padded
at the left edge), runs attention for all 16 heads on its query chunk, and
applies the full output projection locally; outputs concatenate over T.

Layout: feature-major ("transposed") activations. All matmuls run in
float32r at moving-dim >= 256 (full PE rate). Softmax uses exp without
max-subtraction (logits are O(10)), mask via two additive [128,128]
triangle constants, and denominators via an all-ones stationary matmul
that broadcasts column sums to all partitions.
"""
import os
import sys

for _p in ("/root/.axon_site", "/root/.axon_site/_ro/trn_rl_repo", "/opt/trn_rl_repo"):
    if os.path.isdir(_p) and _p not in sys.path:
        sys.path.append(_p)

import numpy as np

import concourse.bass as bass
import concourse.tile as tile
import concourse.mybir as mybir
from concourse.vector_clock import ScopedClock
from concourse.bass_utils import run_bass_kernel_spmd

dt = mybir.dt

B, T, C = 1, 4096, 2048
H, HK, D = 16, 4, 128
W = 1024
THETA = 10000.0
NCORES = 8
TQ = T // NCORES            # 512 queries per core
TK = TQ + W                 # 1536-key span per core
NQC = TQ // 128             # 4 query chunks of 128
NJ = NQC + W // 128 - 3     # 9 key chunks per query chunk
NCT = C // 128              # 16 contraction tiles
SCALE = 1.0 / float(np.sqrt(D))
NEG = -1.0e30


def _patch_tile_drain():
    """CoreV3 codegen caps sync-waits per instruction; the stock TileContext
    tail drain carries one wait per live semaphore.  Spill the waits across
    preceding sync-engine no-ops, one wait each."""
    if getattr(tile.TileContext, "_drain_patched", False):
        return

    def _drain_and_barrier(self, tick_clock, wait_clock):
        nc = self.nc
        probe = nc.sync.nop()
        wait_clock.add_sem_waits(
            probe.ins, ScopedClock({None: tick_clock.global_clock})
        )
        si = probe.ins.sync_info
        waits = list(si.on_wait) if si is not None and si.on_wait else []
        if len(waits) > 1:
            si.on_wait = waits[:1]
            for w in waits[1:]:
                extra = nc.sync.nop()
                extra.ins.sync_info = mybir.SyncInfo(on_wait=[w], on_update=[])
        nc.sync.drain()
        nc.all_engine_barrier()
        assert self.sems is not None
        popped = nc._tile_sem_poison_stack.pop()
        assert popped is self._sem_poison
        nc.clear_and_free_semaphores(list(self.sems.allocated().values()))
        nc.all_engine_barrier()

    tile.TileContext._drain_and_barrier = _drain_and_barrier
    tile.TileContext._drain_patched = True


_MAX_WAITS = 1


def _spill_excess_waits(nc):
    """Walrus codegen caps sync-waits per instruction.  For any instruction
    carrying more, move the excess onto same-engine no-ops inserted just
    before it (engines execute in program order, so the waits still resolve
    before the instruction runs)."""
    n = [0]
    for f in nc.m.functions:
        for bb in f.blocks:
            out = []
            for inst in bb.instructions:
                si = inst.sync_info
                waits = list(si.on_wait) if si is not None and si.on_wait else []
                if len(waits) > _MAX_WAITS:
                    for lo in range(0, len(waits) - _MAX_WAITS, _MAX_WAITS):
                        nop = mybir.InstNoOp(
                            name=f"waitspill-{n[0]}", ins=[], outs=[]
                        )
                        n[0] += 1
                        nop.engine = inst.engine
                        nop.sync_info = mybir.SyncInfo(
                            on_wait=waits[lo:lo + _MAX_WAITS], on_update=[]
                        )
                        out.append(nop)
                    si.on_wait = waits[len(waits) - _MAX_WAITS:]
                out.append(inst)
            bb.instructions[:] = out


def _rope_tables():
    d2 = np.arange(0, D, 2, dtype=np.float64) / D
    ts = THETA ** d2
    ang = np.arange(T, dtype=np.float64)[:, None] / ts[None, :]
    ang = np.concatenate([ang, ang], axis=1)            # [T, D]
    cosT = np.cos(ang).T                                # [D, T]
    sinS = np.sin(ang).T
    sinS[: D // 2] *= -1.0    # rot(u)[d<64] = -u[d+64]; out = u*cos + shift(u)*sinS
    return cosT.astype(np.float32), sinS.astype(np.float32)


def _build_program():
    nc = bass.Bass(num_swdge_queues=4)
    f32, f32r = dt.float32, dt.float32r

    xh_e = nc.declare_dram_parameter("xh", [C, W], f32r, isOutput=False)
    xq_e = nc.declare_dram_parameter("xq", [C, TQ], f32r, isOutput=False)
    wq_e = nc.declare_dram_parameter("wq", [C, H * D], f32r, isOutput=False)
    wk_e = nc.declare_dram_parameter("wk", [C, HK * D], f32r, isOutput=False)
    wv_e = nc.declare_dram_parameter("wv", [C, HK * D], f32r, isOutput=False)
    wo_e = nc.declare_dram_parameter("wo", [H * D, C], f32r, isOutput=False)
    cos_e = nc.declare_dram_parameter("cosk", [D, TK], f32, isOutput=False)
    sin_e = nc.declare_dram_parameter("sink", [D, TK], f32, isOutput=False)
    triw_e = nc.declare_dram_parameter("triw", [128, 128], f32, isOutput=False)
    tric_e = nc.declare_dram_parameter("tric", [128, 128], f32, isOutput=False)
    pad_e = nc.declare_dram_parameter("padfix", [128, NQC * 256], f32, isOutput=False)
    ones_e = nc.declare_dram_parameter("onesc", [128, 128], f32r, isOutput=False)
    y_e = nc.declare_dram_parameter("y", [TQ, C], f32, isOutput=True)

    Exp = mybir.ActivationFunctionType.Exp

    with tile.TileContext(nc) as tc:
        with (
            tc.tile_pool(name="consts", bufs=1) as cst,
            tc.tile_pool(name="vout", bufs=1) as vout,
            tc.tile_pool(name="krp", bufs=1) as krp,
        ):
            cosk = cst.tile([D, TK], f32, tag="cosk")
            sink = cst.tile([D, TK], f32, tag="sink")
            triw = cst.tile([128, 128], f32, tag="triw")
            tric = cst.tile([128, 128], f32, tag="tric")
            padf = cst.tile([128, NQC * 256], f32, tag="padf")
            ones = cst.tile([128, 128], f32r, tag="ones")
            nc.gpsimd.dma_start(cosk[:], cos_e[:])
            nc.gpsimd.dma_start(sink[:], sin_e[:])
            nc.gpsimd.dma_start(triw[:], triw_e[:])
            nc.gpsimd.dma_start(tric[:], tric_e[:])
            nc.gpsimd.dma_start(padf[:], pad_e[:])
            nc.gpsimd.dma_start(ones[:], ones_e[:])

            v_t = [vout.tile([128, HK * D], f32r, name=f"v{tt}", tag=f"v{tt}")
                   for tt in range(TK // 128)]
            kr = [krp.tile([D, TK], f32r, name=f"kr{g}", tag=f"kr{g}")
                  for g in range(HK)]
            an = {}

            # ---------------- projection phases (own PSUM pool) ----------
            pp_cm = tc.tile_pool(name="pp", bufs=4, space="PSUM")
            pp = pp_cm.__enter__()
            xqp_cm = tc.tile_pool(name="xqp", bufs=1)
            xqp = xqp_cm.__enter__()
            xq_t = [xqp.tile([128, TQ], f32r, name=f"xq{ct}", tag=f"xq{ct}")
                    for ct in range(NCT)]
            for ct in range(NCT):
                nc.gpsimd.dma_start(xq_t[ct][:], xq_e[128 * ct:128 * (ct + 1), :])

            xhp_cm = tc.tile_pool(name="xhp", bufs=1)
            xhp = xhp_cm.__enter__()
            xh_t = [xhp.tile([128, W], f32r, name=f"xh{ct}", tag=f"xh{ct}")
                    for ct in range(NCT)]
            for ct in range(NCT):
                nc.gpsimd.dma_start(xh_t[ct][:], xh_e[128 * ct:128 * (ct + 1), :])

            def xk_slice(ct, lo, size):
                # local key cols [lo, lo+size) from history (0..W) / query (W..TK)
                if lo + size <= W:
                    return xh_t[ct][:, lo:lo + size]
                assert lo >= W
                return xq_t[ct][:, lo - W:lo - W + size]

            # ---- V projection: v[t, hd] = x[t, :] @ wv ------------------
            with tc.tile_pool(name="wvp", bufs=1) as wvp:
                wv_t = [wvp.tile([128, HK * D], f32r, name=f"wv{ct}", tag=f"wv{ct}")
                        for ct in range(NCT)]
                for ct in range(NCT):
                    nc.gpsimd.dma_start(wv_t[ct][:], wv_e[128 * ct:128 * (ct + 1), :])
                for tt in range(TK // 128):
                    ps = pp.tile([128, HK * D], dt.float32, tag="pp")
                    for ct in range(NCT):
                        nc.tensor.matmul(
                            ps[:], xk_slice(ct, 128 * tt, 128), wv_t[ct][:],
                            start=(ct == 0), stop=(ct == NCT - 1),
                        )
                    nc.vector.tensor_copy(v_t[tt][:], ps[:])

            # ---- K projection + RoPE: krT[d, t] -------------------------
            with (
                tc.tile_pool(name="wkm", bufs=3) as wkm,
                tc.tile_pool(name="shf", bufs=2) as shf,
            ):
                for g in range(HK):
                    wslab = wkm.tile([128, C], f32r, tag="wkm")
                    src = wk_e[:, 128 * g:128 * (g + 1)].rearrange(
                        "(a p) m -> p a m", p=128
                    )
                    nc.gpsimd.dma_start(
                        wslab[:].rearrange("p (a m) -> p a m", a=NCT), src
                    )
                    for tcb in range(TK // 512):
                        ps = pp.tile([128, 512], dt.float32, tag="pp")
                        for ct in range(NCT):
                            nc.tensor.matmul(
                                ps[:], wslab[:, 128 * ct:128 * (ct + 1)],
                                xk_slice(ct, 512 * tcb, 512),
                                start=(ct == 0), stop=(ct == NCT - 1),
                            )
                        sl = slice(512 * tcb, 512 * (tcb + 1))
                        qs = shf.tile([128, 512], dt.float32, tag="qs")
                        nc.vector.tensor_copy(qs[0:64, :], ps[64:128, :])
                        nc.vector.tensor_copy(qs[64:128, :], ps[0:64, :])
                        nc.vector.tensor_mul(kr[g][:, sl], ps[:], cosk[:, sl])
                        nc.vector.tensor_mul(qs[:], qs[:], sink[:, sl])
                        nc.vector.tensor_add(kr[g][:, sl], kr[g][:, sl], qs[:])

            xhp_cm.__exit__(None, None, None)

            # ---- Q projection + RoPE (query columns only) ---------------
            qpr_cm = tc.tile_pool(name="qpr", bufs=1, side="right")
            qpr = qpr_cm.__enter__()
            qp = [qpr.tile([D, 1024], f32r, name=f"qp{p}", tag=f"qp{p}")
                  for p in range(H // 2)]
            # qp layout: [128, 1024] = 4 qc-blocks of 256 = [even head | odd head]
            with (
                tc.tile_pool(name="wqm", bufs=3) as wqm,
                tc.tile_pool(name="shq", bufs=2) as shq,
            ):
                for m in range(H):
                    pr, half = m // 2, m % 2
                    wslab = wqm.tile([128, C], f32r, tag="wqm")
                    src = wq_e[:, 128 * m:128 * (m + 1)].rearrange(
                        "(a p) m -> p a m", p=128
                    )
                    nc.gpsimd.dma_start(
                        wslab[:].rearrange("p (a m) -> p a m", a=NCT), src
                    )
                    ps = pp.tile([128, TQ], dt.float32, tag="pp")
                    for ct in range(NCT):
                        nc.tensor.matmul(
                            ps[:], wslab[:, 128 * ct:128 * (ct + 1)], xq_t[ct][:],
                            start=(ct == 0), stop=(ct == NCT - 1),
                        )
                    qs = shq.tile([128, TQ], dt.float32, tag="qs")
                    nc.vector.tensor_copy(qs[0:64, :], ps[64:128, :])
                    nc.vector.tensor_copy(qs[64:128, :], ps[0:64, :])
                    csl = slice(W, W + TQ)
                    nc.vector.tensor_mul(qs[:], qs[:], sink[:, csl])
                    for qc in range(NQC):
                        dsl = slice(256 * qc + 128 * half, 256 * qc + 128 * half + 128)
                        ssl = slice(128 * qc, 128 * (qc + 1))
                        nc.vector.tensor_mul(
                            qp[pr][:, dsl], ps[:, ssl],
                            cosk[:, W + 128 * qc:W + 128 * (qc + 1)]
                        )
                        nc.vector.tensor_add(qp[pr][:, dsl], qp[pr][:, dsl], qs[:, ssl])

            xqp_cm.__exit__(None, None, None)
            pp_cm.__exit__(None, None, None)

            # ---- attention ----------------------------------------------
            attnp_cm = tc.tile_pool(name="attn", bufs=1, side="right")
            attnp = attnp_cm.__enter__()
            with (
                tc.tile_pool(name="sc", bufs=4, space="PSUM") as psc,
                tc.tile_pool(name="su", bufs=1, space="PSUM") as psu,
                tc.tile_pool(name="at", bufs=2, space="PSUM") as pat,
                tc.tile_pool(name="pb", bufs=8) as pbp,
                tc.tile_pool(name="rc", bufs=3) as rcp,
                tc.tile_pool(name="yp", bufs=1, space="PSUM") as pyp,
            ):
                for qc in range(NQC):
                    for pr in range(H // 2):
                        g = pr // 2
                        at_ps = pat.tile([128, 256], dt.float32, tag="at")
                        su_ps = psu.tile([128, 256], dt.float32, tag="su")
                        for jp in range((NJ + 1) // 2):
                            njs = 2 if 2 * jp + 1 < NJ else 1
                            sc_ps = psc.tile([128, 512], dt.float32, tag="sc")
                            for s in range(njs):
                                j = 2 * jp + s
                                lk = 128 * (qc + j)
                                hb = 256 * s
                                nc.tensor.matmul(
                                    sc_ps[:, hb:hb + 256], kr[g][:, lk:lk + 128],
                                    qp[pr][:, 256 * qc:256 * (qc + 1)],
                                    start=True, stop=True, skip_group_check=True,
                                )
                                if j == 0:
                                    nc.vector.tensor_add(sc_ps[:, hb:hb + 128], sc_ps[:, hb:hb + 128], triw[:])
                                    nc.vector.tensor_add(sc_ps[:, hb + 128:hb + 256], sc_ps[:, hb + 128:hb + 256], triw[:])
                                if j == NJ - 1:
                                    nc.vector.tensor_add(sc_ps[:, hb:hb + 128], sc_ps[:, hb:hb + 128], tric[:])
                                    nc.vector.tensor_add(sc_ps[:, hb + 128:hb + 256], sc_ps[:, hb + 128:hb + 256], tric[:])
                            pb = pbp.tile([128, 512], f32r, tag="pb")
                            nc.scalar.activation(
                                pb[:, 0:256 * njs], sc_ps[:, 0:256 * njs], Exp, scale=SCALE
                            )
                            for s in range(njs):
                                j = 2 * jp + s
                                hb = 256 * s
                                nc.tensor.matmul(
                                    su_ps[:], ones[:], pb[:, hb:hb + 256],
                                    start=(j == 0), stop=(j == NJ - 1),
                                    skip_group_check=True,
                                )
                                nc.tensor.matmul(
                                    at_ps[:], v_t[qc + j][:, 128 * g:128 * (g + 1)], pb[:, hb:hb + 256],
                                    start=(j == 0), stop=(j == NJ - 1),
                                    skip_group_check=True,
                                )
                        rec = rcp.tile([128, 256], dt.float32, tag="rc")
                        nc.vector.tensor_sub(rec[:], su_ps[:], padf[:, 256 * qc:256 * (qc + 1)])
                        nc.vector.reciprocal(rec[:], rec[:])
                        a = attnp.tile([128, 256], f32r, name=f"an{pr}_{qc}",
                                       tag=f"an{pr}_{qc}")
                        an[(pr, qc)] = a
                        nc.vector.tensor_mul(a[:], at_ps[:], rec[:])

                # ---- output projection (shares PSUM scope) ---------------
                with (
                    tc.tile_pool(name="wop", bufs=16) as wop,
                    tc.tile_pool(name="ych", bufs=4) as ychp,
                ):
                  for cc in range(4):
                    wo_t = []
                    for h in range(H):
                        wt = wop.tile([128, 512], f32r, name=f"wo{h}_{cc}",
                                      tag="wo")
                        nc.gpsimd.dma_start(
                            wt[:], wo_e[128 * h:128 * (h + 1), 512 * cc:512 * (cc + 1)]
                        )
                        wo_t.append(wt)
                    for qc in range(NQC):
                        ps = pyp.tile([128, 512], dt.float32, tag="yp")
                        for h in range(H):
                            pr, half = h // 2, h % 2
                            nc.tensor.matmul(
                                ps[:], an[(pr, qc)][:, 128 * half:128 * (half + 1)],
                                wo_t[h][:],
                                start=(h == 0), stop=(h == H - 1),
                            )
                        ych = ychp.tile([128, 512], dt.float32, tag="ych")
                        nc.scalar.copy(ych[:], ps[:])
                        nc.gpsimd.dma_start(
                            y_e[128 * qc:128 * (qc + 1), 512 * cc:512 * (cc + 1)], ych[:]
                        )
            attnp_cm.__exit__(None, None, None)
            qpr_cm.__exit__(None, None, None)
    _spill_excess_waits(nc)
    return nc


def _host_inputs(x, q_kernel, k_kernel, v_kernel, out_kernel):
    x2 = np.ascontiguousarray(np.asarray(x, np.float32)[0])      # [T, C]
    xT = np.zeros((C, W + T), np.float32)
    xT[:, W:] = x2.T
    cosT, sinS = _rope_tables()
    cos_pad = np.concatenate([np.repeat(cosT[:, :1], W, axis=1), cosT], axis=1)
    sin_pad = np.concatenate([np.repeat(sinS[:, :1], W, axis=1), sinS], axis=1)

    i1 = np.arange(128)
    triw = np.where(i1[None, :] <= i1[:, None], 0.0, NEG).astype(np.float32)  # valid qi <= kj
    tric = np.where(i1[None, :] >= i1[:, None], 0.0, NEG).astype(np.float32)  # valid qi >= kj

    wq = np.ascontiguousarray(np.asarray(q_kernel, np.float32))
    wk = np.ascontiguousarray(np.asarray(k_kernel, np.float32))
    wv = np.ascontiguousarray(np.asarray(v_kernel, np.float32))
    wo = np.ascontiguousarray(np.asarray(out_kernel, np.float32))

    in_maps = []
    for core in range(NCORES):
        q0 = TQ * core
        xk = xT[:, q0:q0 + TK]
        npad = max(0, (W - q0) // 128)
        padf = np.zeros((128, NQC * 256), np.float32)
        qi = np.arange(128, dtype=np.float32)
        for qc in range(NQC):
            pv = np.zeros(128, np.float32)
            if qc < npad:
                pv += 128.0 - qi           # j=0 window chunk: valid count #{kj >= qi}
            for j in range(1, NJ - 1):
                if qc + j < npad:
                    pv += 128.0
            blk = np.tile(pv, 2)           # same for even/odd head halves
            padf[:, 256 * qc:256 * (qc + 1)] = blk[None, :]
        in_maps.append({
            "xh": np.ascontiguousarray(xk[:, :W]),
            "xq": np.ascontiguousarray(xk[:, W:]),
            "wq": wq, "wk": wk, "wv": wv, "wo": wo,
            "cosk": np.ascontiguousarray(cos_pad[:, q0:q0 + TK]),
            "sink": np.ascontiguousarray(sin_pad[:, q0:q0 + TK]),
            "triw": triw, "tric": tric, "padfix": padf,
            "onesc": np.ones((128, 128), np.float32),
        })
    return in_maps


_CACHED = {}


def kernel(x, q_kernel, k_kernel, v_kernel, out_kernel, _profile=False):
    _patch_tile_drain()
    if "nc" not in _CACHED:
        _CACHED["nc"] = _build_program()
    nc = _CACHED["nc"]
    in_maps = _host_inputs(x, q_kernel, k_kernel, v_kernel, out_kernel)
    res = run_bass_kernel_spmd(nc, in_maps, list(range(NCORES)), trace=_profile)
    y = np.concatenate([res.results[i]["y"] for i in range(NCORES)], axis=0)
    out = y[None, :, :].astype(np.float32)
    if _profile:
        return out, res
    return out



# revision 6
# speedup vs baseline: 1.0440x; 1.0440x over previous
"""Trainium2 Bass kernel for block-local (sliding-window) GQA attention with RoPE.

Module: x:[1,4096,2048] -> Q/K/V proj -> RoPE -> block-local attention
(window W=1024, block 1024, GQA 16 q-heads / 4 kv-heads, D=128) -> out proj.

Sharding: sequence-parallel over 8 cores, 512 queries per core. Each core
recomputes K/V for its 1536-row key span (queries + 1024 history, zero-padded
at the left edge), runs attention for all 16 heads on its query chunk, and
applies the full output projection locally; outputs concatenate over T.

Layout: feature-major ("transposed") activations. All matmuls run in
float32r at moving-dim >= 256 (full PE rate). Softmax uses exp without
max-subtraction (logits are O(10)), mask via two additive [128,128]
triangle constants, and denominators via an all-ones stationary matmul
that broadcasts column sums to all partitions.

Scheduling: x history tiles stream in 256-column blocks so V-projection
matmuls start ~18us in; big loads ride the two HWDGE rings (sync: xq/wq/wo,
scalar: x-history/wk/wv/consts) so the Pool engine stays free; wo prefetches
during attention through a 20-buffer pool; softmax reciprocal uses the
fast approximate DVE op; PSUM->SBUF copies run on the scalar engine.
"""
import os
import sys

for _p in ("/root/.axon_site", "/root/.axon_site/_ro/trn_rl_repo", "/opt/trn_rl_repo"):
    if os.path.isdir(_p) and _p not in sys.path:
        sys.path.append(_p)

import numpy as np

import concourse.bass as bass
import concourse.tile as tile
import concourse.mybir as mybir
from concourse.vector_clock import ScopedClock
from concourse.bass_utils import run_bass_kernel_spmd

dt = mybir.dt

B, T, C = 1, 4096, 2048
H, HK, D = 16, 4, 128
W = 1024
THETA = 10000.0
NCORES = 8
TQ = T // NCORES            # 512 queries per core
TK = TQ + W                 # 1536-key span per core
NQC = TQ // 128             # 4 query chunks of 128
NJ = NQC + W // 128 - 3     # 9 key chunks per query chunk
NCT = C // 128              # 16 contraction tiles
SCALE = 1.0 / float(np.sqrt(D))
NEG = -1.0e30


def _patch_tile_drain():
    """CoreV3 codegen caps sync-waits per instruction; the stock TileContext
    tail drain carries one wait per live semaphore.  Spill the waits across
    preceding sync-engine no-ops, one wait each."""
    if getattr(tile.TileContext, "_drain_patched", False):
        return

    def _drain_and_barrier(self, tick_clock, wait_clock):
        nc = self.nc
        probe = nc.sync.nop()
        wait_clock.add_sem_waits(
            probe.ins, ScopedClock({None: tick_clock.global_clock})
        )
        si = probe.ins.sync_info
        waits = list(si.on_wait) if si is not None and si.on_wait else []
        if len(waits) > 1:
            si.on_wait = waits[:1]
            for w in waits[1:]:
                extra = nc.sync.nop()
                extra.ins.sync_info = mybir.SyncInfo(on_wait=[w], on_update=[])
        nc.sync.drain()
        nc.all_engine_barrier()
        assert self.sems is not None
        popped = nc._tile_sem_poison_stack.pop()
        assert popped is self._sem_poison
        nc.clear_and_free_semaphores(list(self.sems.allocated().values()))
        nc.all_engine_barrier()

    tile.TileContext._drain_and_barrier = _drain_and_barrier
    tile.TileContext._drain_patched = True


_MAX_WAITS = 1


def _spill_excess_waits(nc):
    """Walrus codegen caps sync-waits per instruction.  For any instruction
    carrying more, move the excess onto same-engine no-ops inserted just
    before it (engines execute in program order, so the waits still resolve
    before the instruction runs)."""
    n = [0]
    for f in nc.m.functions:
        for bb in f.blocks:
            out = []
            for inst in bb.instructions:
                si = inst.sync_info
                waits = list(si.on_wait) if si is not None and si.on_wait else []
                if len(waits) > _MAX_WAITS:
                    for lo in range(0, len(waits) - _MAX_WAITS, _MAX_WAITS):
                        nop = mybir.InstNoOp(
                            name=f"waitspill-{n[0]}", ins=[], outs=[]
                        )
                        n[0] += 1
                        nop.engine = inst.engine
                        nop.sync_info = mybir.SyncInfo(
                            on_wait=waits[lo:lo + _MAX_WAITS], on_update=[]
                        )
                        out.append(nop)
                    si.on_wait = waits[len(waits) - _MAX_WAITS:]
                out.append(inst)
            bb.instructions[:] = out


def _rope_tables():
    d2 = np.arange(0, D, 2, dtype=np.float64) / D
    ts = THETA ** d2
    ang = np.arange(T, dtype=np.float64)[:, None] / ts[None, :]
    ang = np.concatenate([ang, ang], axis=1)            # [T, D]
    cosT = np.cos(ang).T                                # [D, T]
    sinS = np.sin(ang).T
    sinS[: D // 2] *= -1.0    # rot(u)[d<64] = -u[d+64]; out = u*cos + shift(u)*sinS
    return cosT.astype(np.float32), sinS.astype(np.float32)


def _build_program():
    nc = bass.Bass(num_swdge_queues=4)
    f32, f32r = dt.float32, dt.float32r

    xh_e = nc.declare_dram_parameter("xh", [C, W], f32r, isOutput=False)
    xq_e = nc.declare_dram_parameter("xq", [C, TQ], f32r, isOutput=False)
    wq_e = nc.declare_dram_parameter("wq", [C, H * D], f32r, isOutput=False)
    wk_e = nc.declare_dram_parameter("wk", [C, HK * D], f32r, isOutput=False)
    wv_e = nc.declare_dram_parameter("wv", [C, HK * D], f32r, isOutput=False)
    wo_e = nc.declare_dram_parameter("wo", [H * D, C], f32r, isOutput=False)
    cos_e = nc.declare_dram_parameter("cosk", [D, TK], f32, isOutput=False)
    sin_e = nc.declare_dram_parameter("sink", [D, TK], f32, isOutput=False)
    triw_e = nc.declare_dram_parameter("triw", [128, 128], f32, isOutput=False)
    tric_e = nc.declare_dram_parameter("tric", [128, 128], f32, isOutput=False)
    pad_e = nc.declare_dram_parameter("padfix", [128, NQC * 256], f32, isOutput=False)
    ones_e = nc.declare_dram_parameter("onesc", [128, 128], f32r, isOutput=False)
    y_e = nc.declare_dram_parameter("y", [TQ, C], f32, isOutput=True)

    Exp = mybir.ActivationFunctionType.Exp

    with tile.TileContext(nc) as tc:
        with (
            tc.tile_pool(name="consts", bufs=1) as cst,
            tc.tile_pool(name="vout", bufs=1) as vout,
            tc.tile_pool(name="krp", bufs=1) as krp,
        ):
            cosk = cst.tile([D, TK], f32, tag="cosk")
            sink = cst.tile([D, TK], f32, tag="sink")
            triw = cst.tile([128, 128], f32, tag="triw")
            tric = cst.tile([128, 128], f32, tag="tric")
            padf = cst.tile([128, NQC * 256], f32, tag="padf")
            ones = cst.tile([128, 128], f32r, tag="ones")
            nc.scalar.dma_start(cosk[:], cos_e[:])
            nc.scalar.dma_start(sink[:], sin_e[:])
            nc.scalar.dma_start(triw[:], triw_e[:])
            nc.scalar.dma_start(tric[:], tric_e[:])
            nc.scalar.dma_start(padf[:], pad_e[:])
            nc.scalar.dma_start(ones[:], ones_e[:])

            v_t = [vout.tile([128, HK * D], f32r, name=f"v{tt}", tag=f"v{tt}")
                   for tt in range(TK // 128)]
            kr = [krp.tile([D, TK], f32r, name=f"kr{g}", tag=f"kr{g}")
                  for g in range(HK)]
            an = {}

            # ---------------- projection phases (own PSUM pool) ----------
            pp_cm = tc.tile_pool(name="pp", bufs=4, space="PSUM")
            pp = pp_cm.__enter__()
            xqp_cm = tc.tile_pool(name="xqp", bufs=1)
            xqp = xqp_cm.__enter__()
            xq_t = [xqp.tile([128, TQ], f32r, name=f"xq{ct}", tag=f"xq{ct}")
                    for ct in range(NCT)]
            for ct in range(NCT):
                nc.sync.dma_start(xq_t[ct][:], xq_e[128 * ct:128 * (ct + 1), :])

            xhp_cm = tc.tile_pool(name="xhp", bufs=1)
            xhp = xhp_cm.__enter__()
            xh_t = [xhp.tile([128, W], f32r, name=f"xh{ct}", tag=f"xh{ct}")
                    for ct in range(NCT)]
            # stream xh in 256-wide column blocks so V-proj of the first key
            # chunks starts before the whole history is resident
            for blk in range(W // 256):
                csl = slice(256 * blk, 256 * (blk + 1))
                for ct in range(NCT):
                    nc.scalar.dma_start(
                        xh_t[ct][:, csl], xh_e[128 * ct:128 * (ct + 1), csl]
                    )

            def xk_slice(ct, lo, size):
                # local key cols [lo, lo+size) from history (0..W) / query (W..TK)
                if lo + size <= W:
                    return xh_t[ct][:, lo:lo + size]
                assert lo >= W
                return xq_t[ct][:, lo - W:lo - W + size]

            # ---- V projection: v[t, hd] = x[t, :] @ wv ------------------
            with tc.tile_pool(name="wvp", bufs=1) as wvp:
                wv_t = [wvp.tile([128, HK * D], f32r, name=f"wv{ct}", tag=f"wv{ct}")
                        for ct in range(NCT)]
                for ct in range(NCT):
                    nc.scalar.dma_start(wv_t[ct][:], wv_e[128 * ct:128 * (ct + 1), :])
                for tt in range(TK // 128):
                    ps = pp.tile([128, HK * D], dt.float32, tag="pp")
                    for ct in range(NCT):
                        nc.tensor.matmul(
                            ps[:], xk_slice(ct, 128 * tt, 128), wv_t[ct][:],
                            start=(ct == 0), stop=(ct == NCT - 1),
                        )
                    nc.vector.tensor_copy(v_t[tt][:], ps[:])

            # ---- K projection + RoPE: krT[d, t] -------------------------
            with (
                tc.tile_pool(name="wkm", bufs=3) as wkm,
                tc.tile_pool(name="shf", bufs=2) as shf,
            ):
                for g in range(HK):
                    wslab = wkm.tile([128, C], f32r, tag="wkm")
                    src = wk_e[:, 128 * g:128 * (g + 1)].rearrange(
                        "(a p) m -> p a m", p=128
                    )
                    nc.scalar.dma_start(
                        wslab[:].rearrange("p (a m) -> p a m", a=NCT), src
                    )
                    for tcb in range(TK // 512):
                        ps = pp.tile([128, 512], dt.float32, tag="pp")
                        for ct in range(NCT):
                            nc.tensor.matmul(
                                ps[:], wslab[:, 128 * ct:128 * (ct + 1)],
                                xk_slice(ct, 512 * tcb, 512),
                                start=(ct == 0), stop=(ct == NCT - 1),
                            )
                        sl = slice(512 * tcb, 512 * (tcb + 1))
                        qs = shf.tile([128, 512], dt.float32, tag="qs")
                        nc.vector.tensor_copy(qs[0:64, :], ps[64:128, :])
                        nc.vector.tensor_copy(qs[64:128, :], ps[0:64, :])
                        nc.vector.tensor_mul(kr[g][:, sl], ps[:], cosk[:, sl])
                        nc.vector.tensor_mul(qs[:], qs[:], sink[:, sl])
                        nc.vector.tensor_add(kr[g][:, sl], kr[g][:, sl], qs[:])

            xhp_cm.__exit__(None, None, None)

            # ---- Q projection + RoPE (query columns only) ---------------
            qpr_cm = tc.tile_pool(name="qpr", bufs=1, side="right")
            qpr = qpr_cm.__enter__()
            qp = [qpr.tile([D, 1024], f32r, name=f"qp{p}", tag=f"qp{p}")
                  for p in range(H // 2)]
            # qp layout: [128, 1024] = 4 qc-blocks of 256 = [even head | odd head]
            with (
                tc.tile_pool(name="wqm", bufs=3) as wqm,
                tc.tile_pool(name="shq", bufs=2) as shq,
            ):
                for m in range(H):
                    pr, half = m // 2, m % 2
                    wslab = wqm.tile([128, C], f32r, tag="wqm")
                    src = wq_e[:, 128 * m:128 * (m + 1)].rearrange(
                        "(a p) m -> p a m", p=128
                    )
                    nc.sync.dma_start(
                        wslab[:].rearrange("p (a m) -> p a m", a=NCT), src
                    )
                    ps = pp.tile([128, TQ], dt.float32, tag="pp")
                    for ct in range(NCT):
                        nc.tensor.matmul(
                            ps[:], wslab[:, 128 * ct:128 * (ct + 1)], xq_t[ct][:],
                            start=(ct == 0), stop=(ct == NCT - 1),
                        )
                    qs = shq.tile([128, TQ], dt.float32, tag="qs")
                    nc.vector.tensor_copy(qs[0:64, :], ps[64:128, :])
                    nc.vector.tensor_copy(qs[64:128, :], ps[0:64, :])
                    csl = slice(W, W + TQ)
                    nc.vector.tensor_mul(qs[:], qs[:], sink[:, csl])
                    for qc in range(NQC):
                        dsl = slice(256 * qc + 128 * half, 256 * qc + 128 * half + 128)
                        ssl = slice(128 * qc, 128 * (qc + 1))
                        nc.vector.tensor_mul(
                            qp[pr][:, dsl], ps[:, ssl],
                            cosk[:, W + 128 * qc:W + 128 * (qc + 1)]
                        )
                        nc.vector.tensor_add(qp[pr][:, dsl], qp[pr][:, dsl], qs[:, ssl])

            xqp_cm.__exit__(None, None, None)
            pp_cm.__exit__(None, None, None)

            # ---- attention ----------------------------------------------
            attnp_cm = tc.tile_pool(name="attn", bufs=1, side="right")
            attnp = attnp_cm.__enter__()
            with (
                tc.tile_pool(name="sc", bufs=3, space="PSUM") as psc,
                tc.tile_pool(name="su", bufs=1, space="PSUM") as psu,
                tc.tile_pool(name="at", bufs=2, space="PSUM") as pat,
                tc.tile_pool(name="pb", bufs=6) as pbp,
                tc.tile_pool(name="rc", bufs=2) as rcp,
                tc.tile_pool(name="rc2", bufs=2) as rcp2,
                tc.tile_pool(name="yp", bufs=2, space="PSUM") as pyp,
                tc.tile_pool(name="wop", bufs=20) as wop,
                tc.tile_pool(name="ych", bufs=2) as ychp,
            ):
                # prefetch the whole output-projection weight during attention;
                # bufs=20 double-buffers the 16-tile column rounds
                wo_t = {}
                for cc in range(4):
                    for h in range(H):
                        wt = wop.tile([128, 512], f32r, name=f"wo{h}_{cc}", tag="wo")
                        nc.sync.dma_start(
                            wt[:],
                            wo_e[128 * h:128 * (h + 1), 512 * cc:512 * (cc + 1)],
                        )
                        wo_t[(cc, h)] = wt

                for qc in range(NQC):
                    for pr in range(H // 2):
                        g = pr // 2
                        at_ps = pat.tile([128, 256], dt.float32, tag="at")
                        su_ps = psu.tile([128, 256], dt.float32, tag="su")
                        for jp in range((NJ + 1) // 2):
                            njs = 2 if 2 * jp + 1 < NJ else 1
                            sc_ps = psc.tile([128, 512], dt.float32, tag="sc")
                            for s in range(njs):
                                j = 2 * jp + s
                                lk = 128 * (qc + j)
                                hb = 256 * s
                                nc.tensor.matmul(
                                    sc_ps[:, hb:hb + 256], kr[g][:, lk:lk + 128],
                                    qp[pr][:, 256 * qc:256 * (qc + 1)],
                                    start=True, stop=True, skip_group_check=True,
                                )
                                if j == 0:
                                    nc.vector.tensor_add(sc_ps[:, hb:hb + 128], sc_ps[:, hb:hb + 128], triw[:])
                                    nc.vector.tensor_add(sc_ps[:, hb + 128:hb + 256], sc_ps[:, hb + 128:hb + 256], triw[:])
                                if j == NJ - 1:
                                    nc.vector.tensor_add(sc_ps[:, hb:hb + 128], sc_ps[:, hb:hb + 128], tric[:])
                                    nc.vector.tensor_add(sc_ps[:, hb + 128:hb + 256], sc_ps[:, hb + 128:hb + 256], tric[:])
                            pb = pbp.tile([128, 512], f32r, tag="pb")
                            nc.scalar.activation(
                                pb[:, 0:256 * njs], sc_ps[:, 0:256 * njs], Exp, scale=SCALE
                            )
                            for s in range(njs):
                                j = 2 * jp + s
                                hb = 256 * s
                                nc.tensor.matmul(
                                    su_ps[:], ones[:], pb[:, hb:hb + 256],
                                    start=(j == 0), stop=(j == NJ - 1),
                                    skip_group_check=True,
                                )
                                nc.tensor.matmul(
                                    at_ps[:], v_t[qc + j][:, 128 * g:128 * (g + 1)], pb[:, hb:hb + 256],
                                    start=(j == 0), stop=(j == NJ - 1),
                                    skip_group_check=True,
                                )
                        rec = rcp.tile([128, 256], dt.float32, tag="rc")
                        rec2 = rcp2.tile([128, 256], dt.float32, tag="rc2")
                        nc.vector.tensor_sub(rec[:], su_ps[:], padf[:, 256 * qc:256 * (qc + 1)])
                        nc.vector.reciprocal(rec2[:], rec[:])
                        a = attnp.tile([128, 256], f32r, name=f"an{pr}_{qc}",
                                       tag=f"an{pr}_{qc}")
                        an[(pr, qc)] = a
                        nc.vector.tensor_mul(a[:], at_ps[:], rec2[:])

                # ---- output projection (shares PSUM scope) ---------------
                for cc in range(4):
                    for qc in range(NQC):
                        ps = pyp.tile([128, 512], dt.float32, tag="yp")
                        for h in range(H):
                            pr, half = h // 2, h % 2
                            nc.tensor.matmul(
                                ps[:], an[(pr, qc)][:, 128 * half:128 * (half + 1)],
                                wo_t[(cc, h)][:],
                                start=(h == 0), stop=(h == H - 1),
                            )
                        ych = ychp.tile([128, 512], dt.float32, tag="ych")
                        nc.scalar.copy(ych[:], ps[:])
                        nc.scalar.dma_start(
                            y_e[128 * qc:128 * (qc + 1), 512 * cc:512 * (cc + 1)], ych[:]
                        )
            attnp_cm.__exit__(None, None, None)
            qpr_cm.__exit__(None, None, None)
    _spill_excess_waits(nc)
    return nc


def _host_inputs(x, q_kernel, k_kernel, v_kernel, out_kernel):
    x2 = np.ascontiguousarray(np.asarray(x, np.float32)[0])      # [T, C]
    xT = np.zeros((C, W + T), np.float32)
    xT[:, W:] = x2.T
    cosT, sinS = _rope_tables()
    cos_pad = np.concatenate([np.repeat(cosT[:, :1], W, axis=1), cosT], axis=1)
    sin_pad = np.concatenate([np.repeat(sinS[:, :1], W, axis=1), sinS], axis=1)

    i1 = np.arange(128)
    triw = np.where(i1[None, :] <= i1[:, None], 0.0, NEG).astype(np.float32)  # valid qi <= kj
    tric = np.where(i1[None, :] >= i1[:, None], 0.0, NEG).astype(np.float32)  # valid qi >= kj
    wq = np.ascontiguousarray(np.asarray(q_kernel, np.float32))
    wk = np.ascontiguousarray(np.asarray(k_kernel, np.float32))
    wv = np.ascontiguousarray(np.asarray(v_kernel, np.float32))
    wo = np.ascontiguousarray(np.asarray(out_kernel, np.float32))

    in_maps = []
    for core in range(NCORES):
        q0 = TQ * core
        xk = xT[:, q0:q0 + TK]
        npad = max(0, (W - q0) // 128)
        padf = np.zeros((128, NQC * 256), np.float32)
        qi = np.arange(128, dtype=np.float32)
        for qc in range(NQC):
            pv = np.zeros(128, np.float32)
            if qc < npad:
                pv += 128.0 - qi           # j=0 window chunk: valid count #{kj >= qi}
            for j in range(1, NJ - 1):
                if qc + j < npad:
                    pv += 128.0
            blk = np.tile(pv, 2)           # same for even/odd head halves
            padf[:, 256 * qc:256 * (qc + 1)] = blk[None, :]
        in_maps.append({
            "xh": np.ascontiguousarray(xk[:, :W]),
            "xq": np.ascontiguousarray(xk[:, W:]),
            "wq": wq, "wk": wk, "wv": wv, "wo": wo,
            "cosk": np.ascontiguousarray(cos_pad[:, q0:q0 + TK]),
            "sink": np.ascontiguousarray(sin_pad[:, q0:q0 + TK]),
            "triw": triw, "tric": tric, "padfix": padf,
            "onesc": np.ones((128, 128), np.float32),
        })
    return in_maps


_CACHED = {}


def kernel(x, q_kernel, k_kernel, v_kernel, out_kernel, _profile=False):
    _patch_tile_drain()
    if "nc" not in _CACHED:
        _CACHED["nc"] = _build_program()
    nc = _CACHED["nc"]
    in_maps = _host_inputs(x, q_kernel, k_kernel, v_kernel, out_kernel)
    res = run_bass_kernel_spmd(nc, in_maps, list(range(NCORES)), trace=_profile)
    y = np.concatenate([res.results[i]["y"] for i in range(NCORES)], axis=0)
    out = y[None, :, :].astype(np.float32)
    if _profile:
        return out, res
    return out


# revision 12
# speedup vs baseline: 1.1472x; 1.0989x over previous
"""Trainium2 Bass kernel for block-local (sliding-window) GQA attention with RoPE.

Module: x:[1,4096,2048] -> Q/K/V proj -> RoPE -> block-local attention
(window W=1024, block 1024, GQA 16 q-heads / 4 kv-heads, D=128) -> out proj.

Sharding: sequence-parallel over 8 cores, 512 queries per core. Each core
recomputes K/V for its 1536-row key span (queries + 1024 history, zero-padded
at the left edge), runs attention for all 16 heads on its query chunk, and
applies the full output projection locally; outputs concatenate over T.

Layout: feature-major ("transposed") activations. All matmuls run in
float32r at moving-dim >= 256 (full PE rate). Softmax uses exp without
max-subtraction (logits are O(10)), mask via two additive [128,128]
triangle constants, and denominators via an all-ones stationary matmul
that broadcasts column sums to all partitions.

Scheduling: x history tiles stream in 256-column blocks so V-projection
matmuls start ~18us in; big loads ride the two HWDGE rings (sync: xq/wq/wo,
scalar: x-history/wk/wv/consts) so the Pool engine stays free; wo prefetches
during attention through a 20-buffer pool; softmax reciprocal uses the
fast approximate DVE op; PSUM->SBUF copies run on the scalar engine.
"""
import os
import sys

for _p in ("/root/.axon_site", "/root/.axon_site/_ro/trn_rl_repo", "/opt/trn_rl_repo"):
    if os.path.isdir(_p) and _p not in sys.path:
        sys.path.append(_p)

import numpy as np

import concourse.bass as bass
import concourse.tile as tile
import concourse.mybir as mybir
from concourse.vector_clock import ScopedClock
from concourse.bass_utils import run_bass_kernel_spmd

dt = mybir.dt

B, T, C = 1, 4096, 2048
H, HK, D = 16, 4, 128
W = 1024
THETA = 10000.0
NCORES = 8
TQ = T // NCORES            # 512 queries per core
TK = TQ + W                 # 1536-key span per core
NQC = TQ // 128             # 4 query chunks of 128
NJ = NQC + W // 128 - 3     # 9 key chunks per query chunk
NCT = C // 128              # 16 contraction tiles
SCALE = 1.0 / float(np.sqrt(D))
NEG = -1.0e30


def _patch_tile_drain():
    """CoreV3 codegen caps sync-waits per instruction; the stock TileContext
    tail drain carries one wait per live semaphore.  Spill the waits across
    preceding sync-engine no-ops, one wait each."""
    if getattr(tile.TileContext, "_drain_patched", False):
        return

    def _drain_and_barrier(self, tick_clock, wait_clock):
        nc = self.nc
        probe = nc.sync.nop()
        wait_clock.add_sem_waits(
            probe.ins, ScopedClock({None: tick_clock.global_clock})
        )
        si = probe.ins.sync_info
        waits = list(si.on_wait) if si is not None and si.on_wait else []
        if len(waits) > 1:
            si.on_wait = waits[:1]
            for w in waits[1:]:
                extra = nc.sync.nop()
                extra.ins.sync_info = mybir.SyncInfo(on_wait=[w], on_update=[])
        nc.sync.drain()
        nc.all_engine_barrier()
        assert self.sems is not None
        popped = nc._tile_sem_poison_stack.pop()
        assert popped is self._sem_poison
        nc.clear_and_free_semaphores(list(self.sems.allocated().values()))
        nc.all_engine_barrier()

    tile.TileContext._drain_and_barrier = _drain_and_barrier
    tile.TileContext._drain_patched = True


_MAX_WAITS = 1


def _spill_excess_waits(nc):
    """Walrus codegen caps sync-waits per instruction.  For any instruction
    carrying more, move the excess onto same-engine no-ops inserted just
    before it (engines execute in program order, so the waits still resolve
    before the instruction runs)."""
    n = [0]
    for f in nc.m.functions:
        for bb in f.blocks:
            out = []
            for inst in bb.instructions:
                si = inst.sync_info
                waits = list(si.on_wait) if si is not None and si.on_wait else []
                if len(waits) > _MAX_WAITS:
                    for lo in range(0, len(waits) - _MAX_WAITS, _MAX_WAITS):
                        nop = mybir.InstNoOp(
                            name=f"waitspill-{n[0]}", ins=[], outs=[]
                        )
                        n[0] += 1
                        nop.engine = inst.engine
                        nop.sync_info = mybir.SyncInfo(
                            on_wait=waits[lo:lo + _MAX_WAITS], on_update=[]
                        )
                        out.append(nop)
                    si.on_wait = waits[len(waits) - _MAX_WAITS:]
                out.append(inst)
            bb.instructions[:] = out


def _rope_tables():
    d2 = np.arange(0, D, 2, dtype=np.float64) / D
    ts = THETA ** d2
    ang = np.arange(T, dtype=np.float64)[:, None] / ts[None, :]
    ang = np.concatenate([ang, ang], axis=1)            # [T, D]
    cosT = np.cos(ang).T                                # [D, T]
    sinS = np.sin(ang).T
    sinS[: D // 2] *= -1.0    # rot(u)[d<64] = -u[d+64]; out = u*cos + shift(u)*sinS
    return cosT.astype(np.float32), sinS.astype(np.float32)


def _build_program():
    nc = bass.Bass(num_swdge_queues=4)
    f32, f32r = dt.float32, dt.float32r

    xh_e = nc.declare_dram_parameter("xh", [C, W], f32r, isOutput=False)
    xq_e = nc.declare_dram_parameter("xq", [C, TQ], f32r, isOutput=False)
    wq_e = nc.declare_dram_parameter("wq", [C, H * D], f32r, isOutput=False)
    wk_e = nc.declare_dram_parameter("wk", [C, HK * D], f32r, isOutput=False)
    wv_e = nc.declare_dram_parameter("wv", [C, HK * D], f32r, isOutput=False)
    wo_e = nc.declare_dram_parameter("wo", [H * D, C], f32r, isOutput=False)
    cos_e = nc.declare_dram_parameter("cosk", [D, TK], f32, isOutput=False)
    sin_e = nc.declare_dram_parameter("sink", [D, TK], f32, isOutput=False)
    triw_e = nc.declare_dram_parameter("triw", [128, 128], f32, isOutput=False)
    tric_e = nc.declare_dram_parameter("tric", [128, 128], f32, isOutput=False)
    pad_e = nc.declare_dram_parameter("padfix", [128, NQC * 256], f32, isOutput=False)
    ones_e = nc.declare_dram_parameter("onesc", [128, 128], f32r, isOutput=False)
    y_e = nc.declare_dram_parameter("y", [TQ, C], f32, isOutput=True)

    Exp = mybir.ActivationFunctionType.Exp

    with tile.TileContext(nc) as tc:
        with (
            tc.tile_pool(name="consts", bufs=1) as cst,
            tc.tile_pool(name="vout", bufs=1) as vout,
            tc.tile_pool(name="krp", bufs=1) as krp,
        ):
            cosk = cst.tile([D, TK], f32, tag="cosk")
            sink = cst.tile([D, TK], f32, tag="sink")
            triw = cst.tile([128, 128], f32, tag="triw")
            tric = cst.tile([128, 128], f32, tag="tric")
            padf = cst.tile([128, NQC * 256], f32, tag="padf")
            ones = cst.tile([128, 128], f32r, tag="ones")

            v_t = [vout.tile([128, HK * D], f32r, name=f"v{tt}", tag=f"v{tt}")
                   for tt in range(TK // 128)]
            kr = [krp.tile([D, TK], f32r, name=f"kr{g}", tag=f"kr{g}")
                  for g in range(HK)]
            an = {}

            # ---------------- projection phases (own PSUM pool) ----------
            pp_cm = tc.tile_pool(name="pp", bufs=4, space="PSUM")
            pp = pp_cm.__enter__()
            xqp_cm = tc.tile_pool(name="xqp", bufs=1)
            xqp = xqp_cm.__enter__()
            xq_t = [xqp.tile([128, TQ], f32r, name=f"xq{ct}", tag=f"xq{ct}")
                    for ct in range(NCT)]
            for ct in range(NCT):
                nc.sync.dma_start(xq_t[ct][:], xq_e[128 * ct:128 * (ct + 1), :])

            xhp_cm = tc.tile_pool(name="xhp", bufs=1)
            xhp = xhp_cm.__enter__()
            xh_t = [xhp.tile([128, W], f32r, name=f"xh{ct}", tag=f"xh{ct}")
                    for ct in range(NCT)]

            def xk_slice(ct, lo, size):
                # local key cols [lo, lo+size) from history (0..W) / query (W..TK)
                if lo + size <= W:
                    return xh_t[ct][:, lo:lo + size]
                assert lo >= W
                return xq_t[ct][:, lo - W:lo - W + size]

            # ---- V projection: v[t, hd] = x[t, :] @ wv ------------------
            with tc.tile_pool(name="wvp", bufs=1) as wvp:
                wv_t = [wvp.tile([128, HK * D], f32r, name=f"wv{ct}", tag=f"wv{ct}")
                        for ct in range(NCT)]
                # ring order on the scalar HWDGE queue: wv first (V-proj of
                # the query half starts ~15us in), then RoPE tables, then the
                # 1024-key history block
                for ct in range(NCT):
                    nc.scalar.dma_start(wv_t[ct][:], wv_e[128 * ct:128 * (ct + 1), :])
                nc.scalar.dma_start(cosk[:], cos_e[:])
                nc.scalar.dma_start(sink[:], sin_e[:])
                nc.scalar.dma_start(triw[:], triw_e[:])
                nc.scalar.dma_start(tric[:], tric_e[:])
                nc.scalar.dma_start(padf[:], pad_e[:])
                nc.scalar.dma_start(ones[:], ones_e[:])
                for ct in range(NCT):
                    nc.scalar.dma_start(xh_t[ct][:], xh_e[128 * ct:128 * (ct + 1), :])

                # query-half key chunks first (need only xq + wv), history after
                for tt in list(range(W // 128, TK // 128)) + list(range(W // 128)):
                    ps = pp.tile([128, HK * D], dt.float32, tag="pp")
                    for ct in range(NCT):
                        nc.tensor.matmul(
                            ps[:], xk_slice(ct, 128 * tt, 128), wv_t[ct][:],
                            start=(ct == 0), stop=(ct == NCT - 1),
                        )
                    nc.vector.tensor_copy(v_t[tt][:], ps[:])

            # ---- K projection + RoPE: krT[d, t] -------------------------
            with (
                tc.tile_pool(name="wkm", bufs=3) as wkm,
                tc.tile_pool(name="shf", bufs=2) as shf,
            ):
                for g in range(HK):
                    wslab = wkm.tile([128, C], f32r, tag="wkm")
                    src = wk_e[:, 128 * g:128 * (g + 1)].rearrange(
                        "(a p) m -> p a m", p=128
                    )
                    nc.scalar.dma_start(
                        wslab[:].rearrange("p (a m) -> p a m", a=NCT), src
                    )
                    for tcb in range(TK // 512):
                        ps = pp.tile([128, 512], dt.float32, tag="pp")
                        for ct in range(NCT):
                            nc.tensor.matmul(
                                ps[:], wslab[:, 128 * ct:128 * (ct + 1)],
                                xk_slice(ct, 512 * tcb, 512),
                                start=(ct == 0), stop=(ct == NCT - 1),
                            )
                        sl = slice(512 * tcb, 512 * (tcb + 1))
                        qs = shf.tile([128, 512], dt.float32, tag="qs")
                        nc.vector.tensor_copy(qs[0:64, :], ps[64:128, :])
                        nc.vector.tensor_copy(qs[64:128, :], ps[0:64, :])
                        nc.vector.tensor_mul(kr[g][:, sl], ps[:], cosk[:, sl])
                        nc.vector.tensor_mul(qs[:], qs[:], sink[:, sl])
                        nc.vector.tensor_add(kr[g][:, sl], kr[g][:, sl], qs[:])

            xhp_cm.__exit__(None, None, None)

            # ---- Q projection + RoPE (query columns only) ---------------
            qpr_cm = tc.tile_pool(name="qpr", bufs=1, side="right")
            qpr = qpr_cm.__enter__()
            qp = [qpr.tile([D, 1024], f32r, name=f"qp{p}", tag=f"qp{p}")
                  for p in range(H // 2)]
            # qp layout: [128, 1024] = 4 qc-blocks of 256 = [even head | odd head]
            with (
                tc.tile_pool(name="wqm", bufs=3) as wqm,
                tc.tile_pool(name="shq", bufs=2) as shq,
            ):
                for m in range(H):
                    pr, half = m // 2, m % 2
                    wslab = wqm.tile([128, C], f32r, tag="wqm")
                    src = wq_e[:, 128 * m:128 * (m + 1)].rearrange(
                        "(a p) m -> p a m", p=128
                    )
                    nc.sync.dma_start(
                        wslab[:].rearrange("p (a m) -> p a m", a=NCT), src
                    )
                    ps = pp.tile([128, TQ], dt.float32, tag="pp")
                    for ct in range(NCT):
                        nc.tensor.matmul(
                            ps[:], wslab[:, 128 * ct:128 * (ct + 1)], xq_t[ct][:],
                            start=(ct == 0), stop=(ct == NCT - 1),
                        )
                    qs = shq.tile([128, TQ], dt.float32, tag="qs")
                    nc.vector.tensor_copy(qs[0:64, :], ps[64:128, :])
                    nc.vector.tensor_copy(qs[64:128, :], ps[0:64, :])
                    csl = slice(W, W + TQ)
                    nc.vector.tensor_mul(qs[:], qs[:], sink[:, csl])
                    for qc in range(NQC):
                        dsl = slice(256 * qc + 128 * half, 256 * qc + 128 * half + 128)
                        ssl = slice(128 * qc, 128 * (qc + 1))
                        nc.vector.tensor_mul(
                            qp[pr][:, dsl], ps[:, ssl],
                            cosk[:, W + 128 * qc:W + 128 * (qc + 1)]
                        )
                        nc.vector.tensor_add(qp[pr][:, dsl], qp[pr][:, dsl], qs[:, ssl])

            xqp_cm.__exit__(None, None, None)
            pp_cm.__exit__(None, None, None)

            # ---- attention ----------------------------------------------
            attnp_cm = tc.tile_pool(name="attn", bufs=1, side="right")
            attnp = attnp_cm.__enter__()
            with (
                tc.tile_pool(name="sc", bufs=3, space="PSUM") as psc,
                tc.tile_pool(name="su", bufs=1, space="PSUM") as psu,
                tc.tile_pool(name="at", bufs=2, space="PSUM") as pat,
                tc.tile_pool(name="pb", bufs=6) as pbp,
                tc.tile_pool(name="rc", bufs=2) as rcp,
                tc.tile_pool(name="rc2", bufs=2) as rcp2,
                tc.tile_pool(name="yp", bufs=2, space="PSUM") as pyp,
                tc.tile_pool(name="wop", bufs=20) as wop,
                tc.tile_pool(name="ych", bufs=2) as ychp,
            ):
                # prefetch the whole output-projection weight during attention;
                # bufs=20 double-buffers the 16-tile column rounds
                wo_t = {}
                for cc in range(4):
                    for h in range(H):
                        wt = wop.tile([128, 512], f32r, name=f"wo{h}_{cc}", tag="wo")
                        nc.sync.dma_start(
                            wt[:],
                            wo_e[128 * h:128 * (h + 1), 512 * cc:512 * (cc + 1)],
                        )
                        wo_t[(cc, h)] = wt

                for qc in range(NQC):
                    for pr in range(H // 2):
                        g = pr // 2
                        at_ps = pat.tile([128, 256], dt.float32, tag="at")
                        su_ps = psu.tile([128, 256], dt.float32, tag="su")
                        for jp in range((NJ + 1) // 2):
                            njs = 2 if 2 * jp + 1 < NJ else 1
                            sc_ps = psc.tile([128, 512], dt.float32, tag="sc")
                            for s in range(njs):
                                j = 2 * jp + s
                                lk = 128 * (qc + j)
                                hb = 256 * s
                                nc.tensor.matmul(
                                    sc_ps[:, hb:hb + 256], kr[g][:, lk:lk + 128],
                                    qp[pr][:, 256 * qc:256 * (qc + 1)],
                                    start=True, stop=True, skip_group_check=True,
                                )
                                if j == 0:
                                    nc.vector.tensor_add(sc_ps[:, hb:hb + 128], sc_ps[:, hb:hb + 128], triw[:])
                                    nc.vector.tensor_add(sc_ps[:, hb + 128:hb + 256], sc_ps[:, hb + 128:hb + 256], triw[:])
                                if j == NJ - 1:
                                    nc.vector.tensor_add(sc_ps[:, hb:hb + 128], sc_ps[:, hb:hb + 128], tric[:])
                                    nc.vector.tensor_add(sc_ps[:, hb + 128:hb + 256], sc_ps[:, hb + 128:hb + 256], tric[:])
                            pb = pbp.tile([128, 512], f32r, tag="pb")
                            nc.scalar.activation(
                                pb[:, 0:256 * njs], sc_ps[:, 0:256 * njs], Exp, scale=SCALE
                            )
                            for s in range(njs):
                                j = 2 * jp + s
                                hb = 256 * s
                                nc.tensor.matmul(
                                    su_ps[:], ones[:], pb[:, hb:hb + 256],
                                    start=(j == 0), stop=(j == NJ - 1),
                                    skip_group_check=True,
                                )
                                nc.tensor.matmul(
                                    at_ps[:], v_t[qc + j][:, 128 * g:128 * (g + 1)], pb[:, hb:hb + 256],
                                    start=(j == 0), stop=(j == NJ - 1),
                                    skip_group_check=True,
                                )
                        rec = rcp.tile([128, 256], dt.float32, tag="rc")
                        rec2 = rcp2.tile([128, 256], dt.float32, tag="rc2")
                        nc.vector.tensor_sub(rec[:], su_ps[:], padf[:, 256 * qc:256 * (qc + 1)])
                        nc.vector.reciprocal(rec2[:], rec[:])
                        a = attnp.tile([128, 256], f32r, name=f"an{pr}_{qc}",
                                       tag=f"an{pr}_{qc}")
                        an[(pr, qc)] = a
                        nc.vector.tensor_mul(a[:], at_ps[:], rec2[:])

                # ---- output projection (shares PSUM scope) ---------------
                for cc in range(4):
                    for qc in range(NQC):
                        ps = pyp.tile([128, 512], dt.float32, tag="yp")
                        for h in range(H):
                            pr, half = h // 2, h % 2
                            nc.tensor.matmul(
                                ps[:], an[(pr, qc)][:, 128 * half:128 * (half + 1)],
                                wo_t[(cc, h)][:],
                                start=(h == 0), stop=(h == H - 1),
                            )
                        ych = ychp.tile([128, 512], dt.float32, tag="ych")
                        nc.scalar.copy(ych[:], ps[:])
                        nc.scalar.dma_start(
                            y_e[128 * qc:128 * (qc + 1), 512 * cc:512 * (cc + 1)], ych[:]
                        )
            attnp_cm.__exit__(None, None, None)
            qpr_cm.__exit__(None, None, None)
    _spill_excess_waits(nc)
    return nc


def _host_inputs(x, q_kernel, k_kernel, v_kernel, out_kernel):
    x2 = np.ascontiguousarray(np.asarray(x, np.float32)[0])      # [T, C]
    xT = np.zeros((C, W + T), np.float32)
    xT[:, W:] = x2.T
    cosT, sinS = _rope_tables()
    cos_pad = np.concatenate([np.repeat(cosT[:, :1], W, axis=1), cosT], axis=1)
    sin_pad = np.concatenate([np.repeat(sinS[:, :1], W, axis=1), sinS], axis=1)

    i1 = np.arange(128)
    triw = np.where(i1[None, :] <= i1[:, None], 0.0, NEG).astype(np.float32)  # valid qi <= kj
    tric = np.where(i1[None, :] >= i1[:, None], 0.0, NEG).astype(np.float32)  # valid qi >= kj
    wq = np.ascontiguousarray(np.asarray(q_kernel, np.float32))
    wk = np.ascontiguousarray(np.asarray(k_kernel, np.float32))
    wv = np.ascontiguousarray(np.asarray(v_kernel, np.float32))
    wo = np.ascontiguousarray(np.asarray(out_kernel, np.float32))

    in_maps = []
    for core in range(NCORES):
        q0 = TQ * core
        xk = xT[:, q0:q0 + TK]
        npad = max(0, (W - q0) // 128)
        padf = np.zeros((128, NQC * 256), np.float32)
        qi = np.arange(128, dtype=np.float32)
        for qc in range(NQC):
            pv = np.zeros(128, np.float32)
            if qc < npad:
                pv += 128.0 - qi           # j=0 window chunk: valid count #{kj >= qi}
            for j in range(1, NJ - 1):
                if qc + j < npad:
                    pv += 128.0
            blk = np.tile(pv, 2)           # same for even/odd head halves
            padf[:, 256 * qc:256 * (qc + 1)] = blk[None, :]
        in_maps.append({
            "xh": np.ascontiguousarray(xk[:, :W]),
            "xq": np.ascontiguousarray(xk[:, W:]),
            "wq": wq, "wk": wk, "wv": wv, "wo": wo,
            "cosk": np.ascontiguousarray(cos_pad[:, q0:q0 + TK]),
            "sink": np.ascontiguousarray(sin_pad[:, q0:q0 + TK]),
            "triw": triw, "tric": tric, "padfix": padf,
            "onesc": np.ones((128, 128), np.float32),
        })
    return in_maps


_CACHED = {}


def kernel(x, q_kernel, k_kernel, v_kernel, out_kernel, _profile=False):
    _patch_tile_drain()
    if "nc" not in _CACHED:
        _CACHED["nc"] = _build_program()
    nc = _CACHED["nc"]
    in_maps = _host_inputs(x, q_kernel, k_kernel, v_kernel, out_kernel)
    res = run_bass_kernel_spmd(nc, in_maps, list(range(NCORES)), trace=_profile)
    y = np.concatenate([res.results[i]["y"] for i in range(NCORES)], axis=0)
    out = y[None, :, :].astype(np.float32)
    if _profile:
        return out, res
    return out


# revision 16
# speedup vs baseline: 1.2343x; 1.0759x over previous
"""Trainium2 Bass kernel for block-local (sliding-window) GQA attention with RoPE.

Module: x:[1,4096,2048] -> Q/K/V proj -> RoPE -> block-local attention
(window W=1024, block 1024, GQA 16 q-heads / 4 kv-heads, D=128) -> out proj.

Sharding: sequence-parallel over 8 cores, 512 queries per core. Each core
recomputes K/V for its 1536-row key span (queries + 1024 history, zero-padded
at the left edge), runs attention for all 16 heads on its query chunk, and
applies the full output projection locally; outputs concatenate over T.

Layout: feature-major ("transposed") activations. All matmuls run in
float32r at moving-dim >= 256 (full PE rate). Softmax uses exp without
max-subtraction (logits are O(10)), mask via two additive [128,128]
triangle constants, and denominators via an all-ones stationary matmul
that broadcasts column sums to all partitions.

Scheduling: x history tiles stream in 256-column blocks so V-projection
matmuls start ~18us in; big loads ride the two HWDGE rings (sync: xq/wq/wo,
scalar: x-history/wk/wv/consts) so the Pool engine stays free; wo prefetches
during attention through a 20-buffer pool; softmax reciprocal uses the
fast approximate DVE op; PSUM->SBUF copies run on the scalar engine.
"""
import os
import sys

for _p in ("/root/.axon_site", "/root/.axon_site/_ro/trn_rl_repo", "/opt/trn_rl_repo"):
    if os.path.isdir(_p) and _p not in sys.path:
        sys.path.append(_p)

import numpy as np

import concourse.bass as bass
import concourse.tile as tile
import concourse.mybir as mybir
from concourse.vector_clock import ScopedClock
from concourse.bass_utils import run_bass_kernel_spmd

dt = mybir.dt

B, T, C = 1, 4096, 2048
H, HK, D = 16, 4, 128
W = 1024
THETA = 10000.0
NCORES = 8
TQ = T // NCORES            # 512 queries per core
TK = TQ + W                 # 1536-key span per core
NQC = TQ // 128             # 4 query chunks of 128
NJ = NQC + W // 128 - 3     # 9 key chunks per query chunk
NCT = C // 128              # 16 contraction tiles
SCALE = 1.0 / float(np.sqrt(D))
NEG = -1.0e30


def _patch_tile_drain():
    """CoreV3 codegen caps sync-waits per instruction; the stock TileContext
    tail drain carries one wait per live semaphore.  Spill the waits across
    preceding sync-engine no-ops, one wait each."""
    if getattr(tile.TileContext, "_drain_patched", False):
        return

    def _drain_and_barrier(self, tick_clock, wait_clock):
        nc = self.nc
        probe = nc.sync.nop()
        wait_clock.add_sem_waits(
            probe.ins, ScopedClock({None: tick_clock.global_clock})
        )
        si = probe.ins.sync_info
        waits = list(si.on_wait) if si is not None and si.on_wait else []
        if len(waits) > 1:
            si.on_wait = waits[:1]
            for w in waits[1:]:
                extra = nc.sync.nop()
                extra.ins.sync_info = mybir.SyncInfo(on_wait=[w], on_update=[])
        nc.sync.drain()
        nc.all_engine_barrier()
        assert self.sems is not None
        popped = nc._tile_sem_poison_stack.pop()
        assert popped is self._sem_poison
        nc.clear_and_free_semaphores(list(self.sems.allocated().values()))
        nc.all_engine_barrier()

    tile.TileContext._drain_and_barrier = _drain_and_barrier
    tile.TileContext._drain_patched = True


_MAX_WAITS = 1


def _spill_excess_waits(nc):
    """Walrus codegen caps sync-waits per instruction.  For any instruction
    carrying more, move the excess onto same-engine no-ops inserted just
    before it (engines execute in program order, so the waits still resolve
    before the instruction runs)."""
    n = [0]
    for f in nc.m.functions:
        for bb in f.blocks:
            out = []
            for inst in bb.instructions:
                si = inst.sync_info
                waits = list(si.on_wait) if si is not None and si.on_wait else []
                if len(waits) > _MAX_WAITS:
                    for lo in range(0, len(waits) - _MAX_WAITS, _MAX_WAITS):
                        nop = mybir.InstNoOp(
                            name=f"waitspill-{n[0]}", ins=[], outs=[]
                        )
                        n[0] += 1
                        nop.engine = inst.engine
                        nop.sync_info = mybir.SyncInfo(
                            on_wait=waits[lo:lo + _MAX_WAITS], on_update=[]
                        )
                        out.append(nop)
                    si.on_wait = waits[len(waits) - _MAX_WAITS:]
                out.append(inst)
            bb.instructions[:] = out


def _rope_tables():
    d2 = np.arange(0, D, 2, dtype=np.float64) / D
    ts = THETA ** d2
    ang = np.arange(T, dtype=np.float64)[:, None] / ts[None, :]
    ang = np.concatenate([ang, ang], axis=1)            # [T, D]
    cosT = np.cos(ang).T                                # [D, T]
    sinS = np.sin(ang).T
    sinS[: D // 2] *= -1.0    # rot(u)[d<64] = -u[d+64]; out = u*cos + shift(u)*sinS
    return cosT.astype(np.float32), sinS.astype(np.float32)


def _build_program():
    nc = bass.Bass(num_swdge_queues=4)
    f32, f32r = dt.float32, dt.float32r

    xh_e = nc.declare_dram_parameter("xh", [C, W], f32r, isOutput=False)
    xq_e = nc.declare_dram_parameter("xq", [C, TQ], f32r, isOutput=False)
    wq_e = nc.declare_dram_parameter("wq", [C, H * D], f32r, isOutput=False)
    wk_e = nc.declare_dram_parameter("wk", [C, HK * D], f32r, isOutput=False)
    wv_e = nc.declare_dram_parameter("wv", [C, HK * D], f32r, isOutput=False)
    wo_e = nc.declare_dram_parameter("wo", [H * D, C], f32r, isOutput=False)
    cos_e = nc.declare_dram_parameter("cosk", [D, TK], f32, isOutput=False)
    sin_e = nc.declare_dram_parameter("sink", [D, TK], f32, isOutput=False)
    triw_e = nc.declare_dram_parameter("triw", [128, 128], f32, isOutput=False)
    tric_e = nc.declare_dram_parameter("tric", [128, 128], f32, isOutput=False)
    pad_e = nc.declare_dram_parameter("padfix", [128, NQC * 256], f32, isOutput=False)
    ones_e = nc.declare_dram_parameter("onesc", [128, 128], f32r, isOutput=False)
    y_e = nc.declare_dram_parameter("y", [TQ, C], f32, isOutput=True)

    Exp = mybir.ActivationFunctionType.Exp

    with tile.TileContext(nc) as tc:
        with (
            tc.tile_pool(name="consts", bufs=1) as cst,
            tc.tile_pool(name="vout", bufs=1) as vout,
            tc.tile_pool(name="krp", bufs=1) as krp,
        ):
            cosk = cst.tile([D, TK], f32, tag="cosk")
            sink = cst.tile([D, TK], f32, tag="sink")
            triw = cst.tile([128, 128], f32, tag="triw")
            tric = cst.tile([128, 128], f32, tag="tric")
            padf = cst.tile([128, NQC * 256], f32, tag="padf")
            ones = cst.tile([128, 128], f32r, tag="ones")

            v_t = [vout.tile([128, HK * D], f32r, name=f"v{tt}", tag=f"v{tt}")
                   for tt in range(TK // 128)]
            kr = [krp.tile([D, TK], f32r, name=f"kr{g}", tag=f"kr{g}")
                  for g in range(HK)]
            an = {}

            # ---------------- projection phases (own PSUM pool) ----------
            pp_cm = tc.tile_pool(name="pp", bufs=4, space="PSUM")
            pp = pp_cm.__enter__()
            xqp_cm = tc.tile_pool(name="xqp", bufs=1)
            xqp = xqp_cm.__enter__()
            xq_t = [xqp.tile([128, TQ], f32r, name=f"xq{ct}", tag=f"xq{ct}")
                    for ct in range(NCT)]
            for ct in range(NCT):
                nc.sync.dma_start(xq_t[ct][:], xq_e[128 * ct:128 * (ct + 1), :])

            xhp_cm = tc.tile_pool(name="xhp", bufs=1)
            xhp = xhp_cm.__enter__()
            xh_t = [xhp.tile([128, W], f32r, name=f"xh{ct}", tag=f"xh{ct}")
                    for ct in range(NCT)]

            def xk_slice(ct, lo, size):
                # local key cols [lo, lo+size) from history (0..W) / query (W..TK)
                if lo + size <= W:
                    return xh_t[ct][:, lo:lo + size]
                assert lo >= W
                return xq_t[ct][:, lo - W:lo - W + size]

            # ---- V projection: v[t, hd] = x[t, :] @ wv ------------------
            with tc.tile_pool(name="wvp", bufs=1) as wvp:
                wv_t = [wvp.tile([128, HK * D], f32r, name=f"wv{ct}", tag=f"wv{ct}")
                        for ct in range(NCT)]
                # ring order on the scalar HWDGE queue: wv first (V-proj of
                # the query half starts ~15us in), then RoPE tables, then the
                # 1024-key history block
                for ct in range(NCT):
                    nc.scalar.dma_start(wv_t[ct][:], wv_e[128 * ct:128 * (ct + 1), :])
                nc.scalar.dma_start(cosk[:], cos_e[:])
                nc.scalar.dma_start(sink[:], sin_e[:])
                nc.scalar.dma_start(triw[:], triw_e[:])
                nc.scalar.dma_start(tric[:], tric_e[:])
                nc.scalar.dma_start(padf[:], pad_e[:])
                nc.scalar.dma_start(ones[:], ones_e[:])
                for ct in range(NCT):
                    nc.scalar.dma_start(xh_t[ct][:], xh_e[128 * ct:128 * (ct + 1), :])

                # query-half key chunks first (need only xq + wv), history after
                for tt in list(range(W // 128, TK // 128)) + list(range(W // 128)):
                    ps = pp.tile([128, HK * D], dt.float32, tag="pp")
                    for ct in range(NCT):
                        nc.tensor.matmul(
                            ps[:], xk_slice(ct, 128 * tt, 128), wv_t[ct][:],
                            start=(ct == 0), stop=(ct == NCT - 1),
                        )
                    nc.vector.tensor_copy(v_t[tt][:], ps[:])

            # ---- K projection + RoPE: krT[d, t] -------------------------
            with (
                tc.tile_pool(name="wkm", bufs=3) as wkm,
                tc.tile_pool(name="shf", bufs=2) as shf,
            ):
                for g in range(HK):
                    wslab = wkm.tile([128, C], f32r, tag="wkm")
                    src = wk_e[:, 128 * g:128 * (g + 1)].rearrange(
                        "(a p) m -> p a m", p=128
                    )
                    nc.scalar.dma_start(
                        wslab[:].rearrange("p (a m) -> p a m", a=NCT), src
                    )
                    for tcb in range(TK // 512):
                        ps = pp.tile([128, 512], dt.float32, tag="pp")
                        for ct in range(NCT):
                            nc.tensor.matmul(
                                ps[:], wslab[:, 128 * ct:128 * (ct + 1)],
                                xk_slice(ct, 512 * tcb, 512),
                                start=(ct == 0), stop=(ct == NCT - 1),
                            )
                        sl = slice(512 * tcb, 512 * (tcb + 1))
                        qs = shf.tile([128, 512], dt.float32, tag="qs")
                        # fused rotate-half: qs[p] = ps[p^64] * sinS[p]
                        nc.vector.tensor_mul(qs[0:64, :], ps[64:128, :], sink[0:64, sl])
                        nc.vector.tensor_mul(qs[64:128, :], ps[0:64, :], sink[64:128, sl])
                        nc.vector.tensor_mul(kr[g][:, sl], ps[:], cosk[:, sl])
                        nc.vector.tensor_add(kr[g][:, sl], kr[g][:, sl], qs[:])

            xhp_cm.__exit__(None, None, None)

            # ---- Q projection + RoPE (query columns only) ---------------
            qpr_cm = tc.tile_pool(name="qpr", bufs=1, side="right")
            qpr = qpr_cm.__enter__()
            qp = [qpr.tile([D, 1024], f32r, name=f"qp{p}", tag=f"qp{p}")
                  for p in range(H // 2)]
            # qp layout: [128, 1024] = 4 qc-blocks of 256 = [even head | odd head]
            with (
                tc.tile_pool(name="wqm", bufs=3) as wqm,
                tc.tile_pool(name="shq", bufs=2) as shq,
            ):
                for m in range(H):
                    pr, half = m // 2, m % 2
                    wslab = wqm.tile([128, C], f32r, tag="wqm")
                    src = wq_e[:, 128 * m:128 * (m + 1)].rearrange(
                        "(a p) m -> p a m", p=128
                    )
                    nc.sync.dma_start(
                        wslab[:].rearrange("p (a m) -> p a m", a=NCT), src
                    )
                    ps = pp.tile([128, TQ], dt.float32, tag="pp")
                    for ct in range(NCT):
                        nc.tensor.matmul(
                            ps[:], wslab[:, 128 * ct:128 * (ct + 1)], xq_t[ct][:],
                            start=(ct == 0), stop=(ct == NCT - 1),
                        )
                    qs = shq.tile([128, TQ], dt.float32, tag="qs")
                    csl = slice(W, W + TQ)
                    # fused rotate-half: qs[p] = ps[p^64] * sinS[p]
                    nc.vector.tensor_mul(qs[0:64, :], ps[64:128, :], sink[0:64, csl])
                    nc.vector.tensor_mul(qs[64:128, :], ps[0:64, :], sink[64:128, csl])
                    # strided dest view: qp half-lanes of all 4 qc blocks at once
                    dst = qp[pr][:].rearrange("p (a b) -> p a b", b=256)[
                        :, :, 128 * half:128 * (half + 1)
                    ]
                    ps_v = ps[:].rearrange("p (a b) -> p a b", b=128)
                    qs_v = qs[:].rearrange("p (a b) -> p a b", b=128)
                    cos_v = cosk[:, csl].rearrange("p (a b) -> p a b", b=128)
                    nc.vector.tensor_mul(dst, ps_v, cos_v)
                    nc.vector.tensor_add(dst, dst, qs_v)

            xqp_cm.__exit__(None, None, None)
            pp_cm.__exit__(None, None, None)

            # ---- attention ----------------------------------------------
            attnp_cm = tc.tile_pool(name="attn", bufs=1, side="right")
            attnp = attnp_cm.__enter__()
            with (
                tc.tile_pool(name="sc", bufs=3, space="PSUM") as psc,
                tc.tile_pool(name="su", bufs=1, space="PSUM") as psu,
                tc.tile_pool(name="at", bufs=2, space="PSUM") as pat,
                tc.tile_pool(name="pb", bufs=6) as pbp,
                tc.tile_pool(name="rc", bufs=2) as rcp,
                tc.tile_pool(name="rc2", bufs=2) as rcp2,
                tc.tile_pool(name="yp", bufs=2, space="PSUM") as pyp,
                tc.tile_pool(name="wop", bufs=20) as wop,
                tc.tile_pool(name="ych", bufs=2) as ychp,
            ):
                # prefetch the whole output-projection weight during attention;
                # bufs=20 double-buffers the 16-tile column rounds
                wo_t = {}
                for cc in range(4):
                    for h in range(H):
                        wt = wop.tile([128, 512], f32r, name=f"wo{h}_{cc}", tag="wo")
                        nc.sync.dma_start(
                            wt[:],
                            wo_e[128 * h:128 * (h + 1), 512 * cc:512 * (cc + 1)],
                        )
                        wo_t[(cc, h)] = wt

                # wide attention: one 512-query-column moving operand covers a
                # pair of query chunks; each key chunk in the pair's union
                # band is one scores/denominator/AV matmul triple at the full
                # fp32r rate.  A 2-iteration skew keeps the PE busy while the
                # mask adds + exp for a chunk run on DVE/Act.
                NKJ = NJ + 1          # key chunks per query-chunk pair
                for P in range(NQC // 2):
                    q0 = 2 * P
                    for pr in range(H // 2):
                        g = pr // 2
                        at_ps = pat.tile([128, 512], dt.float32, tag="at")
                        su_ps = psu.tile([128, 512], dt.float32, tag="su")
                        sc_l, pb_l = {}, {}
                        for idx in range(NKJ + 2):
                            if idx < NKJ:
                                kj = q0 + idx
                                sc_ps = psc.tile([128, 512], dt.float32, tag="sc")
                                sc_l[idx] = sc_ps
                                nc.tensor.matmul(
                                    sc_ps[:], kr[g][:, 128 * kj:128 * (kj + 1)],
                                    qp[pr][:, 512 * P:512 * (P + 1)],
                                    start=True, stop=True, skip_group_check=True,
                                )
                                for s, qc in enumerate((q0, q0 + 1)):
                                    cb = 256 * s
                                    j = kj - qc
                                    if j < 0 or j > NJ - 1:
                                        nc.vector.tensor_scalar_add(
                                            sc_ps[:, cb:cb + 256],
                                            sc_ps[:, cb:cb + 256], NEG,
                                        )
                                    else:
                                        if j == 0:
                                            nc.vector.tensor_add(sc_ps[:, cb:cb + 128], sc_ps[:, cb:cb + 128], triw[:])
                                            nc.vector.tensor_add(sc_ps[:, cb + 128:cb + 256], sc_ps[:, cb + 128:cb + 256], triw[:])
                                        if j == NJ - 1:
                                            nc.vector.tensor_add(sc_ps[:, cb:cb + 128], sc_ps[:, cb:cb + 128], tric[:])
                                            nc.vector.tensor_add(sc_ps[:, cb + 128:cb + 256], sc_ps[:, cb + 128:cb + 256], tric[:])
                                pb = pbp.tile([128, 512], f32r, tag="pb")
                                pb_l[idx] = pb
                                nc.scalar.activation(pb[:], sc_ps[:], Exp, scale=SCALE)
                            if idx >= 2:
                                k = idx - 2
                                pb = pb_l.pop(k)
                                kj = q0 + k
                                nc.tensor.matmul(
                                    su_ps[:], ones[:], pb[:],
                                    start=(k == 0), stop=(k == NKJ - 1),
                                    skip_group_check=True,
                                )
                                nc.tensor.matmul(
                                    at_ps[:], v_t[kj][:, 128 * g:128 * (g + 1)], pb[:],
                                    start=(k == 0), stop=(k == NKJ - 1),
                                    skip_group_check=True,
                                )
                        rec = rcp.tile([128, 512], dt.float32, tag="rc")
                        rec2 = rcp2.tile([128, 512], dt.float32, tag="rc2")
                        nc.vector.tensor_sub(rec[:], su_ps[:], padf[:, 512 * P:512 * (P + 1)])
                        nc.vector.reciprocal(rec2[:], rec[:])
                        a = attnp.tile([128, 512], f32r, name=f"an{pr}_{P}",
                                       tag=f"an{pr}_{P}")
                        an[(pr, P)] = a
                        nc.vector.tensor_mul(a[:], at_ps[:], rec2[:])

                # ---- output projection (shares PSUM scope) ---------------
                for cc in range(4):
                    for qc in range(NQC):
                        ps = pyp.tile([128, 512], dt.float32, tag="yp")
                        for h in range(H):
                            pr, half = h // 2, h % 2
                            cb = 256 * (qc % 2) + 128 * half
                            nc.tensor.matmul(
                                ps[:], an[(pr, qc // 2)][:, cb:cb + 128],
                                wo_t[(cc, h)][:],
                                start=(h == 0), stop=(h == H - 1),
                            )
                        ych = ychp.tile([128, 512], dt.float32, tag="ych")
                        nc.scalar.copy(ych[:], ps[:])
                        nc.scalar.dma_start(
                            y_e[128 * qc:128 * (qc + 1), 512 * cc:512 * (cc + 1)], ych[:]
                        )
            attnp_cm.__exit__(None, None, None)
            qpr_cm.__exit__(None, None, None)
    _spill_excess_waits(nc)
    return nc


def _host_inputs(x, q_kernel, k_kernel, v_kernel, out_kernel):
    x2 = np.ascontiguousarray(np.asarray(x, np.float32)[0])      # [T, C]
    xT = np.zeros((C, W + T), np.float32)
    xT[:, W:] = x2.T
    cosT, sinS = _rope_tables()
    cos_pad = np.concatenate([np.repeat(cosT[:, :1], W, axis=1), cosT], axis=1)
    sin_pad = np.concatenate([np.repeat(sinS[:, :1], W, axis=1), sinS], axis=1)

    i1 = np.arange(128)
    triw = np.where(i1[None, :] <= i1[:, None], 0.0, NEG).astype(np.float32)  # valid qi <= kj
    tric = np.where(i1[None, :] >= i1[:, None], 0.0, NEG).astype(np.float32)  # valid qi >= kj
    wq = np.ascontiguousarray(np.asarray(q_kernel, np.float32))
    wk = np.ascontiguousarray(np.asarray(k_kernel, np.float32))
    wv = np.ascontiguousarray(np.asarray(v_kernel, np.float32))
    wo = np.ascontiguousarray(np.asarray(out_kernel, np.float32))

    in_maps = []
    for core in range(NCORES):
        q0 = TQ * core
        xk = xT[:, q0:q0 + TK]
        npad = max(0, (W - q0) // 128)
        padf = np.zeros((128, NQC * 256), np.float32)
        qi = np.arange(128, dtype=np.float32)
        for qc in range(NQC):
            pv = np.zeros(128, np.float32)
            if qc < npad:
                pv += 128.0 - qi           # j=0 window chunk: valid count #{kj >= qi}
            for j in range(1, NJ - 1):
                if qc + j < npad:
                    pv += 128.0
            blk = np.tile(pv, 2)           # same for even/odd head halves
            padf[:, 256 * qc:256 * (qc + 1)] = blk[None, :]
        in_maps.append({
            "xh": np.ascontiguousarray(xk[:, :W]),
            "xq": np.ascontiguousarray(xk[:, W:]),
            "wq": wq, "wk": wk, "wv": wv, "wo": wo,
            "cosk": np.ascontiguousarray(cos_pad[:, q0:q0 + TK]),
            "sink": np.ascontiguousarray(sin_pad[:, q0:q0 + TK]),
            "triw": triw, "tric": tric, "padfix": padf,
            "onesc": np.ones((128, 128), np.float32),
        })
    return in_maps


_CACHED = {}


def kernel(x, q_kernel, k_kernel, v_kernel, out_kernel, _profile=False):
    _patch_tile_drain()
    if "nc" not in _CACHED:
        _CACHED["nc"] = _build_program()
    nc = _CACHED["nc"]
    in_maps = _host_inputs(x, q_kernel, k_kernel, v_kernel, out_kernel)
    res = run_bass_kernel_spmd(nc, in_maps, list(range(NCORES)), trace=_profile)
    y = np.concatenate([res.results[i]["y"] for i in range(NCORES)], axis=0)
    out = y[None, :, :].astype(np.float32)
    if _profile:
        return out, res
    return out
